# revision 8
# baseline (speedup 1.0000x reference)
"""AttentionPooling (segment softmax-pool) Trainium2 kernel — v4 (bf16).

out[s,:] = sum_n 1[idx[n]==s] * gnorm[n] * (x[n,:] @ msg_w + msg_b)
  gnorm[n] = w[n]^p * exp(gate[n]) / (denom[seg] + eps)   (max-sub skipped:
  exact after normalization, logits are O(6))

Reformulation: fold gate_w into x on the host (x' = x*gw,
msg_w' = msg_w/gw — diagonal reparametrization), ship p*ln(w) as an
extra column.  Per-row logit per window (T tiles of 128 rows):

  tile row = [x*gw (128) | ones | p*ln(w) | pad pad] bf16 ++ mask u8 (32)
  h        = x'[0:64] + x'[64:128]      (TT, bf16 2x mode)
  logit_x  = reduce_sum(h)              (DVE 1x, half width)
  logit    = (plw + 1) + logit_x        (STT; +1/gate_b cancel in softmax)
  tail tile's logit via ACT accum (Copy over cols 0:130) to balance engines
  gex      = exp(logit)                 (ACT, f32)
  G        = mask_u8 * gex              (broadcast TT on DVE + a few ACT tiles)
  psA[h]  += G^T @ row[0:129]           (bf16 matmul, M=32, PSUM f32)

Windows are 32 segments; TWO windows stack in one [64, 129] PSUM tile
(partition offsets 0/32) so phase 2 runs once per pair (bf16):
  rcp = 1/(den+eps); Anorm = psA*rcp (ACT; col 128 -> coef = den*rcp);
  ps2 = AnormT^T @ msg_w' + coefT^T(K=1) @ msg_b;  out = copy(ps2).

Sharding: index is sorted; 2048 contiguous segments per core, 64
windows/core; per-window tile counts are the max over the 8 cores
(one SPMD program), derived from the actual index at first call.
"""

import os
import sys
import numpy as np

for _p in ("/opt/trn_rl_repo", "/root/.axon_site/_ro/trn_rl_repo"):
    if os.path.isdir(_p) and _p not in sys.path:
        sys.path.insert(0, _p)

P = 128
S = 16384
D = 128
NCORES = 8
WIN = 32                       # segments per PSUM half-window
NWIN = S // WIN                # 512 global windows
NWIN_CORE = NWIN // NCORES     # 64 per core
CONE = D                       # ones column (kept adjacent to x' for the matmul)
CPLW = D + 1                   # plw column
NBF = D + 2                    # bf16 cols per tile (130; RB stays 4B-aligned)
RB = 2 * NBF + WIN             # merged row bytes per tile: 264 bf16 + 32 u8
CW = RB // 2                   # bf16 view cols per tile (148)
EPS = 1e-10

ACT_G_TILES = 6                # leading tiles per pair G-built on ACT
K_TAIL = 2                     # trailing tiles per pair logit-reduced on ACT

LAST_EXEC_NS = None
LAST_RESULTS = None

_module_cache = {}


def _build_module(t_slots):
    key = ("v5", t_slots, ACT_G_TILES, K_TAIL)
    if key in _module_cache:
        return _module_cache[key]

    import concourse.bass as bass  # noqa: F401
    import concourse.tile as tile
    from concourse import bacc, mybir
    from concourse.masks import make_identity

    f32 = mybir.dt.float32
    bf16 = mybir.dt.bfloat16
    u8 = mybir.dt.uint8
    AX = mybir.AxisListType
    ALU = mybir.AluOpType
    ACTF = mybir.ActivationFunctionType

    NPAIR = NWIN_CORE // 2
    t_pair = [t_slots[2 * p] + t_slots[2 * p + 1] for p in range(NPAIR)]
    tmax = max(t_pair)

    nc = bacc.Bacc(
        "TRN2",
        target_bir_lowering=False,
        debug=False,
        enable_asserts=True,
        num_devices=NCORES,
    )

    xm = nc.dram_tensor("xm", [NPAIR * P, tmax * RB], u8,
                        kind="ExternalInput")
    msgwp = nc.dram_tensor("msgwp", [D, D], bf16, kind="ExternalInput")
    msgbrow = nc.dram_tensor("msgbrow", [1, D], bf16, kind="ExternalInput")
    out = nc.dram_tensor("out", [NWIN_CORE * WIN, D], f32, kind="ExternalOutput")

    with tile.TileContext(nc) as tc:
        from contextlib import ExitStack

        with ExitStack() as ctx:
            const_pool = ctx.enter_context(tc.tile_pool(name="const", bufs=1))
            xs_pool = ctx.enter_context(tc.tile_pool(name="xs", bufs=6))
            grp_pool = ctx.enter_context(tc.tile_pool(name="grp", bufs=8))
            g_pool = ctx.enter_context(tc.tile_pool(name="gm", bufs=4))
            psA_pool = ctx.enter_context(tc.tile_pool(name="psA", bufs=2, space="PSUM"))
            ps2_pool = ctx.enter_context(tc.tile_pool(name="ps2", bufs=2, space="PSUM"))
            ph2_pool = ctx.enter_context(tc.tile_pool(name="ph2", bufs=3))

            msgw_t = const_pool.tile([D, D], bf16)
            nc.sync.dma_start(msgw_t[:], msgwp[:, :])
            msgb_t = const_pool.tile([1, D], bf16)
            nc.sync.dma_start(msgb_t[:], msgbrow[:, :])
            ident_bf = const_pool.tile([2 * WIN, 2 * WIN], bf16)
            make_identity(nc, ident_bf[:])

            chains = {}
            gmats = {}

            def emit_chain(w):
                T = t_pair[w]
                nT = T - K_TAIL
                xt = xs_pool.tile([P, T * RB], u8, tag="xs", name=f"xs{w}")
                nc.sync.dma_start(xt[:], xm[w * P : (w + 1) * P, 0 : T * RB])
                xb = xt[:].bitcast(bf16)                       # [P, T*CW]
                xb3 = xb.rearrange("p (j c) -> p j c", c=CW)
                logit = grp_pool.tile([P, T], f32, tag="logit", name=f"lg{w}")
                # DVE path: two 2x pair-fold levels, then quarter-width reduce
                h = grp_pool.tile([P, nT * 64], bf16, tag="h", name=f"h{w}")
                h3 = h[:].rearrange("p (j c) -> p j c", c=64)
                nc.vector.tensor_tensor(
                    out=h3,
                    in0=xb3[:, 0:nT, 0:64],
                    in1=xb3[:, 0:nT, 64:D],
                    op=ALU.add,
                )
                h2 = grp_pool.tile([P, nT * 32], bf16, tag="h2", name=f"h2{w}")
                h23 = h2[:].rearrange("p (j c) -> p j c", c=32)
                nc.vector.tensor_tensor(
                    out=h23,
                    in0=h3[:, :, 0:32],
                    in1=h3[:, :, 32:64],
                    op=ALU.add,
                )
                lx = grp_pool.tile([P, nT], f32, tag="lx", name=f"lx{w}")
                nc.vector.reduce_sum(out=lx[:], in_=h23, axis=AX.X)
                nc.vector.scalar_tensor_tensor(
                    out=logit[:, 0:nT],
                    in0=xb3[:, 0:nT, CPLW],
                    scalar=1.0,
                    in1=lx[:],
                    op0=ALU.add,
                    op1=ALU.add,
                )
                # ACT path for tail tiles: accum over [x'|plw|ones] = logit+1
                for j in range(nT, T):
                    junk = grp_pool.tile([P, CPLW + 1], bf16, tag="junk",
                                         name=f"jk{w}_{j}")
                    nc.scalar.activation(
                        out=junk[:],
                        in_=xb[:, j * CW : j * CW + CPLW + 1],
                        func=ACTF.Copy,
                        accum_out=logit[:, j : j + 1],
                    )
                gex = grp_pool.tile([P, T], f32, tag="gex", name=f"gx{w}")
                nc.scalar.activation(out=gex[:], in_=logit[:], func=ACTF.Exp)
                chains[w] = (xt, gex)

            def emit_G(w):
                T = t_pair[w]
                xt, gex = chains.pop(w)
                mk3 = xt[:].rearrange("p (j b) -> p j b", b=RB)
                Gt = g_pool.tile([P, T * WIN], bf16, tag="G", name=f"G{w}")
                G3 = Gt[:].rearrange("p (j s) -> p j s", s=WIN)
                a = min(ACT_G_TILES, T)
                for j in range(a):
                    nc.scalar.activation(
                        out=G3[:, j, :],
                        in_=mk3[:, j, 2 * NBF : RB],
                        func=ACTF.Copy,
                        scale=gex[:, j : j + 1],
                    )
                if a < T:
                    gexB = gex[:, a:T].unsqueeze(2).broadcast_to([P, T - a, WIN])
                    nc.vector.tensor_tensor(
                        out=G3[:, a:T, :],
                        in0=mk3[:, a:T, 2 * NBF : RB],
                        in1=gexB,
                        op=ALU.mult,
                    )
                gmats[w] = (xt, Gt)

            def emit_mms(w, psA):
                T = t_pair[w]
                T0 = t_slots[2 * w]
                xt, Gt = gmats.pop(w)
                xb = xt[:].bitcast(bf16)
                for j in range(T):
                    half = int(j >= T0)
                    nc.tensor.matmul(
                        out=psA[half * WIN : (half + 1) * WIN, :],
                        lhsT=Gt[:, j * WIN : (j + 1) * WIN],
                        rhs=xb[:, j * CW : j * CW + D + 1],
                        start=(j == 0 or j == T0),
                        stop=(j == T0 - 1 or j == T - 1),
                    )

            def emit_phase2(pair, psA):
                deno = ph2_pool.tile([2 * WIN, 1], f32, tag="deno", name=f"dn{pair}")
                nc.vector.tensor_scalar_add(deno[:], psA[:, D : D + 1], EPS)
                rcp = ph2_pool.tile([2 * WIN, 1], f32, tag="rcp", name=f"rc{pair}")
                nc.vector.reciprocal(out=rcp[:], in_=deno[:])
                # Anorm = psA * rcp ; col 128 becomes coef = den/(den+eps)
                anorm = ph2_pool.tile([2 * WIN, D + 1], bf16, tag="anorm",
                                      name=f"an{pair}")
                nc.scalar.activation(
                    out=anorm[:], in_=psA[:], func=ACTF.Copy, scale=rcp[:, 0:1]
                )
                psAT = ps2_pool.tile([P, 2 * WIN], bf16, tag="AT", name=f"AT{pair}")
                nc.tensor.transpose(
                    out=psAT[:], in_=anorm[:, 0:D], identity=ident_bf[:]
                )
                sbAT = ph2_pool.tile([P, 2 * WIN], bf16, tag="sbAT", name=f"sT{pair}")
                nc.scalar.activation(out=sbAT[:], in_=psAT[:], func=ACTF.Copy)
                psCT = ps2_pool.tile([1, 2 * WIN], bf16, tag="CT", name=f"CT{pair}")
                nc.tensor.transpose(
                    out=psCT[:], in_=anorm[:, D : D + 1], identity=ident_bf[:]
                )
                sbCT = ph2_pool.tile([1, 2 * WIN], bf16, tag="sbCT", name=f"sC{pair}")
                nc.vector.tensor_copy(sbCT[:], psCT[:])
                ps2 = ps2_pool.tile([2 * WIN, D], f32, tag="out2", name=f"o2{pair}")
                nc.tensor.matmul(
                    out=ps2[:], lhsT=sbAT[:], rhs=msgw_t[:], start=True, stop=False
                )
                nc.tensor.matmul(
                    out=ps2[:], lhsT=sbCT[:], rhs=msgb_t[:], start=False, stop=True
                )
                ofin = ph2_pool.tile([2 * WIN, D], f32, tag="ofin", name=f"of{pair}")
                nc.scalar.activation(out=ofin[:], in_=ps2[:], func=ACTF.Copy)
                nc.sync.dma_start(
                    out[pair * 2 * WIN : (pair + 1) * 2 * WIN, :], ofin[:]
                )

            emit_chain(0)
            emit_chain(1)
            emit_G(0)
            for p in range(NPAIR):
                if p + 2 < NPAIR:
                    emit_chain(p + 2)
                if p + 1 < NPAIR:
                    emit_G(p + 1)
                psA = psA_pool.tile([2 * WIN, D + 1], f32, tag="psA",
                                    name=f"psA{p}")
                emit_mms(p, psA)
                emit_phase2(p, psA)

    nc.compile()
    _module_cache[key] = nc
    return nc


def _prep_inputs(x, idx, w, gw, pw):
    """Fold gate_w/pow into merged per-core [x|mask] device rows."""
    import ml_dtypes

    bf = ml_dtypes.bfloat16
    n = idx.shape[0]
    bounds = np.searchsorted(idx, np.arange(0, S + 1, WIN)).astype(np.int64)
    counts = np.diff(bounds)                          # [NWIN] rows per window
    t_real = (counts + P - 1) // P                    # tiles per global window
    # one SPMD program: per-window-slot tile count = max over the 8 cores
    t_slots = tuple(int(v) for v in t_real.reshape(NCORES, NWIN_CORE).max(axis=0))
    tpw = np.tile(np.asarray(t_slots, np.int64), NCORES)   # padded tiles per window
    row_cap = tpw * P
    wstart = np.concatenate([[0], np.cumsum(row_cap)])[:-1]  # padded row offset

    dest = np.arange(n, dtype=np.int64) + np.repeat(wstart - bounds[:-1], counts)

    total_rows = int(row_cap.sum())
    xpad = np.zeros((total_rows, NBF), dtype=bf)
    xpad[dest, 0:D] = (x.astype(np.float64) * gw[None, :]).astype(bf)
    xpad[dest, CPLW] = (pw * np.log(w.astype(np.float64))).astype(bf)
    xpad[dest, CONE] = np.float32(1.0)

    segl = idx - np.repeat(np.arange(NWIN, dtype=np.int64) * WIN, counts)
    mask = np.zeros((total_rows, WIN), dtype=np.uint8)
    mask[dest, segl] = 1

    # merged rows: [130 bf16 | 32 u8] per tile, gathered to device layout;
    # two windows concatenate into one device row (pair processing)
    merged = np.concatenate([xpad.view(np.uint8), mask], axis=1)  # [rows, RB]
    npair = NWIN_CORE // 2
    t_pair = [t_slots[2 * p] + t_slots[2 * p + 1] for p in range(npair)]
    tmax = max(t_pair)
    xdev = np.zeros((NCORES, npair * P, tmax * RB), dtype=np.uint8)
    rows_per_core = total_rows // NCORES
    for c in range(NCORES):
        off = c * rows_per_core
        for pi in range(npair):
            T = t_pair[pi]
            blk = merged[off : off + T * P].reshape(T, P, RB)
            xdev[c, pi * P : (pi + 1) * P, 0 : T * RB] = (
                blk.transpose(1, 0, 2).reshape(P, T * RB)
            )
            off += T * P
    return xdev, t_slots


def _ensure_ntff_hook():
    """The image's antenv package lacks axon_hooks; shim it so trace=True
    can register the ctypes NTFF hook from trn_agent_boot."""
    try:
        from antenv.axon_hooks import get_axon_ntff_profile_hook  # noqa: F401

        return True
    except ImportError:
        pass
    try:
        import types

        import antenv
        from trn_agent_boot.trn_boot import _ntff_profile_via_ctypes

        mod = types.ModuleType("antenv.axon_hooks")
        _hook = [None]
        mod.set_axon_ntff_profile_hook = lambda h: _hook.__setitem__(0, h)
        mod.get_axon_ntff_profile_hook = lambda: _hook[0]
        sys.modules["antenv.axon_hooks"] = mod
        antenv.axon_hooks = mod
        mod.set_axon_ntff_profile_hook(
            _ntff_profile_via_ctypes("/opt/axon/libaxon_pjrt.so")
        )
        return True
    except Exception as e:  # degrade to untraced run
        print(f"ntff hook install failed: {type(e).__name__}: {e}")
        return False


def kernel(x, index, weights, gate_w, gate_b, msg_w, msg_b, pow_p):
    global LAST_EXEC_NS, LAST_RESULTS

    x = np.ascontiguousarray(np.asarray(x, dtype=np.float32))
    idx = np.asarray(index).astype(np.int64).ravel()
    w = np.asarray(weights, dtype=np.float32).ravel()
    gate_w = np.asarray(gate_w, dtype=np.float64).reshape(D)
    msg_w = np.ascontiguousarray(np.asarray(msg_w, dtype=np.float64))
    msg_b = np.asarray(msg_b, dtype=np.float32).reshape(D)
    pw = float(np.asarray(pow_p, dtype=np.float64).reshape(1)[0])
    # gate_b shifts every logit equally -> cancels in the segment softmax.

    if not np.all(idx[1:] >= idx[:-1]):
        perm = np.argsort(idx, kind="stable")
        idx = idx[perm]
        x = x[perm]
        w = w[perm]

    xdev, t_slots = _prep_inputs(x, idx, w, gate_w, pw)

    import ml_dtypes

    bf = ml_dtypes.bfloat16
    msgwp = np.ascontiguousarray((msg_w / gate_w[:, None]).astype(bf))
    msgbrow = np.asarray(msg_b, np.float32).reshape(1, D).astype(bf)

    nc = _build_module(t_slots)
    from concourse.bass_utils import run_bass_kernel_spmd

    in_maps = []
    for c in range(NCORES):
        in_maps.append(
            {
                "xm": np.ascontiguousarray(xdev[c]),
                "msgwp": msgwp,
                "msgbrow": msgbrow,
            }
        )

    trace = bool(os.environ.get("KERNEL_TRACE"))
    if trace:
        trace = _ensure_ntff_hook()
    res = run_bass_kernel_spmd(
        nc, in_maps, core_ids=list(range(NCORES)), trace=trace
    )
    LAST_RESULTS = res
    LAST_EXEC_NS = res.exec_time_ns

    out = np.concatenate([res.results[c]["out"] for c in range(NCORES)], axis=0)
    return out.astype(np.float32)


def kernel_numpy(x, index, weights, gate_w, gate_b, msg_w, msg_b, pow_p):
    """Host-side mirror of the device algorithm (debug only)."""
    import ml_dtypes

    bf = ml_dtypes.bfloat16
    x = np.asarray(x, dtype=np.float64)
    idx = np.asarray(index).astype(np.int64).ravel()
    w = np.asarray(weights, dtype=np.float64).ravel()
    gw = np.asarray(gate_w, dtype=np.float64).reshape(D)
    pw = float(np.asarray(pow_p, dtype=np.float64).reshape(1)[0])
    xg = (x * gw[None, :]).astype(bf).astype(np.float32)
    plw = (pw * np.log(w)).astype(bf).astype(np.float32)
    h = (xg[:, 0:64] + xg[:, 64:128]).astype(bf).astype(np.float32)
    logit = plw + h.sum(axis=1, dtype=np.float32) + 1.0
    G = np.exp(logit).astype(np.float32).astype(bf).astype(np.float32)
    A = np.zeros((S, D), np.float32)
    den = np.zeros(S, np.float32)
    np.add.at(A, idx, G[:, None] * xg)
    np.add.at(den, idx, G)
    rcp = (1.0 / (den + EPS)).astype(np.float32)
    msgwp = (np.asarray(msg_w, np.float64) / gw[:, None]).astype(bf).astype(
        np.float32
    )
    anorm = (A * rcp[:, None]).astype(bf).astype(np.float32)
    coef = (den * rcp).astype(bf).astype(np.float32)
    out = anorm @ msgwp + coef[:, None] * np.asarray(msg_b, np.float32).reshape(
        1, D
    ).astype(np.float32)
    return out.astype(np.float32)
